# revision 24
# baseline (speedup 1.0000x reference)
"""Trainium2 Bass kernel for nn_GroupedLinear (16-group LayerNorm+Linear).

Problem: x [1024, 8, 64, 64] fp32; per group g (16 groups of 64 channels):
  X_g = contiguous 2M-element chunk g viewed row-major as [32768, 64]
  Y_g = LayerNorm(X_g) * gamma_g + beta_g  @ W_g^T + b_g      [32768, 64]
  out chunk g = Y_g^T  (contiguous [64, 32768] block of the output)

Sharding: expert-parallel, 2 groups per core across 8 cores; no collectives.

v4: bf16 wire format both ways; host pre-interleaves the two groups
channel-wise (x_prep[row, (c,g)]) so every device-side access pattern is
contiguous: bn_stats reads [p,128] unit-stride (even/odd stream = g0/g1),
the normalize runs in DVE 2x mode, transposes read contiguous stripes, and
the weight matrix is row-permuted to match the (c,g) contraction order.
Output is written in PSUM column order (contiguous bf16) and the host
inverts the known column permutation during unshard. Engine split per
2048-row macro: DVE = 16x bn_stats + rstd-multiply (2x mode) + one
psum->sbuf job; Pool = mean subtract; ACT = rstd/mu prep + remaining
psum->sbuf copies; PE = 16 transposes + 2 N=1024 matmuls.
"""

import sys

for _p in ("/opt/trn_rl_repo", "/opt/pypackages"):
    if _p not in sys.path:
        sys.path.insert(0, _p)

import numpy as np
import ml_dtypes

G_TOTAL = 16
N_CORES = 8
G_PER_CORE = G_TOTAL // N_CORES  # 2
IN_G = 64
OUT_G = 64
K = G_PER_CORE * IN_G  # 128 interleaved (c,g) channels
ROWS = 8 * 64 * 64  # 32768 rows per group
MACRO = 4096  # rows per macro-tile
NB = MACRO // 128  # 16 row-blocks per macro (row = p*NB + b)
NMAC = ROWS // MACRO  # 16
EPS = 1e-6

_CACHE = {}
_LAST_RESULTS = None


def _build_bass(rep=1):
    import concourse.bacc as bacc
    import concourse.bass as bass
    import concourse.tile as tile
    from concourse import mybir

    nc = bacc.Bacc(None, target_bir_lowering=False)

    x = nc.dram_tensor("x", [ROWS, K], mybir.dt.bfloat16,
                       kind="ExternalInput")
    wb = nc.dram_tensor("wb", [128, 128], mybir.dt.bfloat16,
                        kind="ExternalInput")
    tb = nc.dram_tensor("tb", [128, 1], mybir.dt.float32,
                        kind="ExternalInput")
    ident = nc.dram_tensor("ident", [128, 128], mybir.dt.bfloat16,
                           kind="ExternalInput")
    out = nc.dram_tensor("out", [128, ROWS], mybir.dt.bfloat16,
                         kind="ExternalOutput")

    F = mybir.ActivationFunctionType

    with tile.TileContext(nc) as tc:
        with (
            tc.tile_pool(name="singles", bufs=1) as singles,
            tc.tile_pool(name="xload", bufs=4) as xload,
            tc.tile_pool(name="statp", bufs=4) as statp,
            tc.tile_pool(name="rstdp", bufs=4) as rstdp,
            tc.tile_pool(name="xnp", bufs=3) as xnp,
            tc.tile_pool(name="xtsp", bufs=4) as xtsp,
            tc.tile_pool(name="youtp", bufs=3) as youtp,
            tc.tile_pool(name="xtpp", bufs=2, space="PSUM") as xtpp,
            tc.tile_pool(name="ypp", bufs=3, space="PSUM") as ypp,
        ):
            sb_wb = singles.tile([128, 128], mybir.dt.bfloat16)
            sb_tb = singles.tile([128, 1], mybir.dt.float32)
            sb_id = singles.tile([128, 128], mybir.dt.bfloat16)
            sb_eps = singles.tile([128, 1], mybir.dt.float32)
            nc.sync.dma_start(out=sb_wb, in_=wb[:, :])
            nc.sync.dma_start(out=sb_tb, in_=tb[:, :])
            nc.sync.dma_start(out=sb_id, in_=ident[:, :])
            nc.vector.memset(sb_eps, EPS)

            def stage_a(m):
                r0 = (m % NMAC) * MACRO
                # load: partition p holds rows NB*p .. NB*p+NB-1, all 128
                # interleaved channels -> one 4KB contiguous run/partition
                x_t = xload.tile([128, NB, K], mybir.dt.bfloat16)
                nc.sync.dma_start(
                    out=x_t,
                    in_=x[r0:r0 + MACRO, :].rearrange(
                        "(p b) k -> p b k", p=128),
                )
                # stats: 16 bn_stats on contiguous [p, 128] streams; (c,g)
                # interleave -> even positions g0, odd g1; out [p, 6] =
                # [cnt0, mu0, M2_0, cnt1, mu1, M2_1]
                st = statp.tile([128, NB, 6], mybir.dt.float32)
                for bb in range(NB):
                    nc.vector.add_instruction(
                        mybir.InstBNStats(
                            name=nc.get_next_instruction_name(),
                            ins=[nc.vector.lower_ap(x_t[:, bb, :])],
                            outs=[nc.vector.lower_ap(st[:, bb, :])],
                        )
                    )
                # rstd2[p, b, g] = 1/sqrt(M2/64+eps) bf16 (g contiguous)
                rstd2 = rstdp.tile([128, NB, G_PER_CORE], mybir.dt.bfloat16)
                st_ap = st[:, :, :]
                m2_in = bass.AP(
                    tensor=st_ap.tensor, offset=st_ap.offset + 2,
                    ap=[st_ap.ap[0], [6, NB], [3, G_PER_CORE]],
                )
                nc.scalar.activation(out=rstd2, in_=m2_in,
                                     func=F.Abs_reciprocal_sqrt,
                                     bias=sb_eps[:, 0:1],
                                     scale=1.0 / IN_G)
                # mean subtract on GpSimd (fp32 mu broadcast)
                xc = xnp.tile([128, NB, K], mybir.dt.bfloat16)
                xn_bcg = bass.AP(
                    tensor=xc.tensor, offset=xc[:, :, :].offset,
                    ap=[xc[:, :, :].ap[0], [K, NB], [2, IN_G], [1, 2]],
                )
                xt_bcg = bass.AP(
                    tensor=x_t.tensor, offset=x_t[:, :, :].offset,
                    ap=[x_t[:, :, :].ap[0], [K, NB], [2, IN_G], [1, 2]],
                )
                mu_b = bass.AP(
                    tensor=st_ap.tensor, offset=st_ap.offset + 1,
                    ap=[st_ap.ap[0], [6, NB], [0, IN_G], [3, G_PER_CORE]],
                )
                nc.gpsimd.tensor_sub(xn_bcg, xt_bcg, mu_b)
                return (m % NMAC, xc, rstd2)

            def stage_b(p):
                m, xc, rstd2 = p
                r0 = m * MACRO
                # rstd multiply on DVE in 2x mode (all-bf16, unit-stride
                # innermost g-pairs), in place
                xn = xc
                xn_bcg = bass.AP(
                    tensor=xn.tensor, offset=xn[:, :, :].offset,
                    ap=[xn[:, :, :].ap[0], [K, NB], [2, IN_G], [1, 2]],
                )
                rstd_b = bass.AP(
                    tensor=rstd2.tensor, offset=rstd2[:, :, :].offset,
                    ap=[rstd2[:, :, :].ap[0], [2, NB], [0, IN_G], [1, 2]],
                )
                nc.vector.tensor_mul(xn_bcg, xn_bcg, rstd_b)
                # per half: 8 transposes -> PSUM, ACT copy -> SBUF, 2
                # N=512 matmuls -> PSUM f32, bias-add -> contiguous bf16
                # (PSUM column order; host inverts the permutation)
                y_t = youtp.tile([128, MACRO], mybir.dt.bfloat16)
                for h in range(NB // 8):
                    xtp = xtpp.tile([128, 1024], mybir.dt.bfloat16)
                    for s in range(8):
                        nc.tensor.transpose(
                            out=xtp[:, s * 128:(s + 1) * 128],
                            in_=xn[:, 8 * h + s, :],
                            identity=sb_id,
                        )
                    xts = xtsp.tile([128, 1024], mybir.dt.bfloat16)
                    nc.scalar.activation(out=xts, in_=xtp, func=F.Copy)
                    yp = ypp.tile([128, 1024], mybir.dt.float32)
                    for j in range(2):
                        nc.tensor.matmul(yp[:, j * 512:(j + 1) * 512],
                                         lhsT=sb_wb,
                                         rhs=xts[:, j * 512:(j + 1) * 512],
                                         start=True, stop=True)
                    yt_v = y_t[:, h * 1024:(h + 1) * 1024]
                    nc.scalar.activation(out=yt_v, in_=yp,
                                         func=F.Identity,
                                         bias=sb_tb[:, 0:1], scale=1.0)
                # out-DMA on the scalar HWDGE ring so the sync ring's FIFO
                # (carrying the input loads) never blocks behind it
                nc.scalar.dma_start(out=out[:, r0:r0 + MACRO], in_=y_t)

            for m in range(NMAC * rep):
                stage_b(stage_a(m))

    nc.finalize()
    return nc


def _get_nc(rep=1):
    key = ("nc", rep)
    if key not in _CACHE:
        _CACHE[key] = _build_bass(rep)
    return _CACHE[key]


def _make_in_maps(x, ln_gamma, ln_beta, W, b):
    bf16 = ml_dtypes.bfloat16
    xg = x.reshape(G_TOTAL, ROWS, IN_G)
    ident_bf = np.eye(128, dtype=np.float32).astype(bf16)
    # channel permutation: device k = c*2 + g  <- source (g, c)
    perm = np.empty(128, np.int64)
    for g in range(G_PER_CORE):
        for c in range(IN_G):
            perm[c * G_PER_CORE + g] = g * IN_G + c
    in_maps = []
    for core in range(N_CORES):
        gs = [G_PER_CORE * core + g for g in range(G_PER_CORE)]
        wbc = np.zeros((128, 128), np.float32)
        tbc = np.zeros((128, 1), np.float32)
        for g_local, g in enumerate(gs):
            Wp = W[g] * ln_gamma[g][None, :]  # [out, in] gamma folded
            lo = g_local * 64
            wbc[lo:lo + 64, lo:lo + 64] = Wp.T  # lhsT[k=in, m=out]
            tbc[lo:lo + 64, 0] = W[g] @ ln_beta[g] + b[g]
        wbc = wbc[perm, :]  # rows now in interleaved (c,g) order
        # x interleaved: [ROWS, (c,g)]
        xi = np.ascontiguousarray(
            xg[gs[0]:gs[-1] + 1].transpose(1, 2, 0).reshape(ROWS, K)
        ).astype(bf16)
        in_maps.append({
            "x": xi,
            "wb": wbc.astype(bf16),
            "tb": tbc,
            "ident": ident_bf,
        })
    return in_maps


def _unpermute(dev_out):
    """Invert the PSUM column order: flat = m*2048 + h*1024 + s*128 + q
    maps to row m*2048 + q*16 + 8h + s."""
    a = dev_out.reshape(128, NMAC, NB // 8, 8, 128)  # [p, m, h, s, q]
    a = a.transpose(0, 1, 4, 2, 3)  # [p, m, q, h, s]
    return np.ascontiguousarray(a).reshape(128, ROWS)


def _run(in_maps, trace=False):
    from concourse.bass_utils import run_bass_kernel_spmd
    global _LAST_RESULTS
    nc = _get_nc()
    res = run_bass_kernel_spmd(nc, in_maps, list(range(N_CORES)),
                               trace=trace)
    _LAST_RESULTS = res
    return res


def kernel(x, ln_gamma, ln_beta, W, b):
    x = np.asarray(x, np.float32)
    ln_gamma = np.asarray(ln_gamma, np.float32)
    ln_beta = np.asarray(ln_beta, np.float32)
    W = np.asarray(W, np.float32)
    b = np.asarray(b, np.float32)

    in_maps = _make_in_maps(x, ln_gamma, ln_beta, W, b)
    res = _run(in_maps, trace=False)
    outs = [_unpermute(np.asarray(r["out"])).astype(np.float32)
            for r in res.results]
    full = np.concatenate(outs, axis=0)  # [1024, 32768]
    return full.reshape(1024, 8, 64, 64)


# revision 25
# speedup vs baseline: 1.1572x; 1.1572x over previous
"""Trainium2 Bass kernel for nn_GroupedLinear (16-group LayerNorm+Linear).

Problem: x [1024, 8, 64, 64] fp32; per group g (16 groups of 64 channels):
  X_g = contiguous 2M-element chunk g viewed row-major as [32768, 64]
  Y_g = LayerNorm(X_g) * gamma_g + beta_g  @ W_g^T + b_g      [32768, 64]
  out chunk g = Y_g^T  (contiguous [64, 32768] block of the output)

Sharding: expert-parallel, 2 groups per core across 8 cores; no collectives.

v4: bf16 wire format both ways; host pre-interleaves the two groups
channel-wise (x_prep[row, (c,g)]) so every device-side access pattern is
contiguous: bn_stats reads [p,128] unit-stride (even/odd stream = g0/g1),
the normalize runs in DVE 2x mode, transposes read contiguous stripes, and
the weight matrix is row-permuted to match the (c,g) contraction order.
Output is written in PSUM column order (contiguous bf16) and the host
inverts the known column permutation during unshard. Engine split per
2048-row macro: DVE = 16x bn_stats + rstd-multiply (2x mode) + one
psum->sbuf job; Pool = mean subtract; ACT = rstd/mu prep + remaining
psum->sbuf copies; PE = 16 transposes + 2 N=1024 matmuls.
"""

import sys

for _p in ("/opt/trn_rl_repo", "/opt/pypackages"):
    if _p not in sys.path:
        sys.path.insert(0, _p)

import numpy as np
import ml_dtypes

G_TOTAL = 16
N_CORES = 8
G_PER_CORE = G_TOTAL // N_CORES  # 2
IN_G = 64
OUT_G = 64
K = G_PER_CORE * IN_G  # 128 interleaved (c,g) channels
ROWS = 8 * 64 * 64  # 32768 rows per group
MACRO = 2048  # rows per macro-tile
NB = MACRO // 128  # 16 row-blocks per macro (row = p*NB + b)
NMAC = ROWS // MACRO  # 16
EPS = 1e-6

_CACHE = {}
_LAST_RESULTS = None


def _build_bass(rep=1):
    import concourse.bacc as bacc
    import concourse.bass as bass
    import concourse.tile as tile
    from concourse import mybir

    nc = bacc.Bacc(None, target_bir_lowering=False)

    x = nc.dram_tensor("x", [ROWS, K], mybir.dt.bfloat16,
                       kind="ExternalInput")
    wb = nc.dram_tensor("wb", [128, 128], mybir.dt.bfloat16,
                        kind="ExternalInput")
    tb = nc.dram_tensor("tb", [128, 1], mybir.dt.float32,
                        kind="ExternalInput")
    ident = nc.dram_tensor("ident", [128, 128], mybir.dt.bfloat16,
                           kind="ExternalInput")
    out = nc.dram_tensor("out", [128, ROWS], mybir.dt.bfloat16,
                         kind="ExternalOutput")

    F = mybir.ActivationFunctionType

    with tile.TileContext(nc) as tc:
        with (
            tc.tile_pool(name="singles", bufs=1) as singles,
            tc.tile_pool(name="xload", bufs=4) as xload,
            tc.tile_pool(name="statp", bufs=4) as statp,
            tc.tile_pool(name="rstdp", bufs=4) as rstdp,
            tc.tile_pool(name="xnp", bufs=3) as xnp,
            tc.tile_pool(name="xtsp", bufs=4) as xtsp,
            tc.tile_pool(name="youtp", bufs=3) as youtp,
            tc.tile_pool(name="xtpp", bufs=2, space="PSUM") as xtpp,
            tc.tile_pool(name="ypp", bufs=3, space="PSUM") as ypp,
        ):
            sb_wb = singles.tile([128, 128], mybir.dt.bfloat16)
            sb_tb = singles.tile([128, 1], mybir.dt.float32)
            sb_id = singles.tile([128, 128], mybir.dt.bfloat16)
            sb_eps = singles.tile([128, 1], mybir.dt.float32)
            nc.sync.dma_start(out=sb_wb, in_=wb[:, :])
            nc.sync.dma_start(out=sb_tb, in_=tb[:, :])
            nc.sync.dma_start(out=sb_id, in_=ident[:, :])
            nc.vector.memset(sb_eps, EPS)

            def stage_a(m):
                r0 = (m % NMAC) * MACRO
                # load: partition p holds rows NB*p .. NB*p+NB-1, all 128
                # interleaved channels -> one 4KB contiguous run/partition
                x_t = xload.tile([128, NB, K], mybir.dt.bfloat16)
                nc.sync.dma_start(
                    out=x_t,
                    in_=x[r0:r0 + MACRO, :].rearrange(
                        "(p b) k -> p b k", p=128),
                )
                # stats: 16 bn_stats on contiguous [p, 128] streams; (c,g)
                # interleave -> even positions g0, odd g1; out [p, 6] =
                # [cnt0, mu0, M2_0, cnt1, mu1, M2_1]
                st = statp.tile([128, NB, 6], mybir.dt.float32)
                for bb in range(NB):
                    nc.vector.add_instruction(
                        mybir.InstBNStats(
                            name=nc.get_next_instruction_name(),
                            ins=[nc.vector.lower_ap(x_t[:, bb, :])],
                            outs=[nc.vector.lower_ap(st[:, bb, :])],
                        )
                    )
                # rstd2[p, b, g] = 1/sqrt(M2/64+eps) bf16 (g contiguous)
                rstd2 = rstdp.tile([128, NB, G_PER_CORE], mybir.dt.bfloat16)
                st_ap = st[:, :, :]
                m2_in = bass.AP(
                    tensor=st_ap.tensor, offset=st_ap.offset + 2,
                    ap=[st_ap.ap[0], [6, NB], [3, G_PER_CORE]],
                )
                nc.scalar.activation(out=rstd2, in_=m2_in,
                                     func=F.Abs_reciprocal_sqrt,
                                     bias=sb_eps[:, 0:1],
                                     scale=1.0 / IN_G)
                # mean subtract on GpSimd (fp32 mu broadcast)
                xc = xnp.tile([128, NB, K], mybir.dt.bfloat16)
                xn_bcg = bass.AP(
                    tensor=xc.tensor, offset=xc[:, :, :].offset,
                    ap=[xc[:, :, :].ap[0], [K, NB], [2, IN_G], [1, 2]],
                )
                xt_bcg = bass.AP(
                    tensor=x_t.tensor, offset=x_t[:, :, :].offset,
                    ap=[x_t[:, :, :].ap[0], [K, NB], [2, IN_G], [1, 2]],
                )
                mu_b = bass.AP(
                    tensor=st_ap.tensor, offset=st_ap.offset + 1,
                    ap=[st_ap.ap[0], [6, NB], [0, IN_G], [3, G_PER_CORE]],
                )
                nc.gpsimd.tensor_sub(xn_bcg, xt_bcg, mu_b)
                return (m % NMAC, xc, rstd2)

            def stage_b(p):
                m, xc, rstd2 = p
                r0 = m * MACRO
                # rstd multiply on DVE in 2x mode (all-bf16, unit-stride
                # innermost g-pairs), in place
                xn = xc
                xn_bcg = bass.AP(
                    tensor=xn.tensor, offset=xn[:, :, :].offset,
                    ap=[xn[:, :, :].ap[0], [K, NB], [2, IN_G], [1, 2]],
                )
                rstd_b = bass.AP(
                    tensor=rstd2.tensor, offset=rstd2[:, :, :].offset,
                    ap=[rstd2[:, :, :].ap[0], [2, NB], [0, IN_G], [1, 2]],
                )
                nc.vector.tensor_mul(xn_bcg, xn_bcg, rstd_b)
                # per half: 8 transposes -> PSUM, ACT copy -> SBUF, 2
                # N=512 matmuls -> PSUM f32, bias-add -> contiguous bf16
                # (PSUM column order; host inverts the permutation)
                y_t = youtp.tile([128, MACRO], mybir.dt.bfloat16)
                for h in range(2):
                    xtp = xtpp.tile([128, 1024], mybir.dt.bfloat16)
                    for s in range(8):
                        nc.tensor.transpose(
                            out=xtp[:, s * 128:(s + 1) * 128],
                            in_=xn[:, 8 * h + s, :],
                            identity=sb_id,
                        )
                    xts = xtsp.tile([128, 1024], mybir.dt.bfloat16)
                    nc.scalar.activation(out=xts, in_=xtp, func=F.Copy)
                    yp = ypp.tile([128, 1024], mybir.dt.float32)
                    for j in range(2):
                        nc.tensor.matmul(yp[:, j * 512:(j + 1) * 512],
                                         lhsT=sb_wb,
                                         rhs=xts[:, j * 512:(j + 1) * 512],
                                         start=True, stop=True)
                    yt_v = y_t[:, h * 1024:(h + 1) * 1024]
                    nc.scalar.activation(out=yt_v, in_=yp,
                                         func=F.Identity,
                                         bias=sb_tb[:, 0:1], scale=1.0)
                # out-DMA on the scalar HWDGE ring so the sync ring's FIFO
                # (carrying the input loads) never blocks behind it
                nc.scalar.dma_start(out=out[:, r0:r0 + MACRO], in_=y_t)

            for m in range(NMAC * rep):
                stage_b(stage_a(m))

    nc.finalize()
    return nc


def _get_nc(rep=1):
    key = ("nc", rep)
    if key not in _CACHE:
        _CACHE[key] = _build_bass(rep)
    return _CACHE[key]


def _make_in_maps(x, ln_gamma, ln_beta, W, b):
    bf16 = ml_dtypes.bfloat16
    xg = x.reshape(G_TOTAL, ROWS, IN_G)
    ident_bf = np.eye(128, dtype=np.float32).astype(bf16)
    # channel permutation: device k = c*2 + g  <- source (g, c)
    perm = np.empty(128, np.int64)
    for g in range(G_PER_CORE):
        for c in range(IN_G):
            perm[c * G_PER_CORE + g] = g * IN_G + c
    in_maps = []
    for core in range(N_CORES):
        gs = [G_PER_CORE * core + g for g in range(G_PER_CORE)]
        wbc = np.zeros((128, 128), np.float32)
        tbc = np.zeros((128, 1), np.float32)
        for g_local, g in enumerate(gs):
            Wp = W[g] * ln_gamma[g][None, :]  # [out, in] gamma folded
            lo = g_local * 64
            wbc[lo:lo + 64, lo:lo + 64] = Wp.T  # lhsT[k=in, m=out]
            tbc[lo:lo + 64, 0] = W[g] @ ln_beta[g] + b[g]
        wbc = wbc[perm, :]  # rows now in interleaved (c,g) order
        # x interleaved: [ROWS, (c,g)]
        xi = np.ascontiguousarray(
            xg[gs[0]:gs[-1] + 1].transpose(1, 2, 0).reshape(ROWS, K)
        ).astype(bf16)
        in_maps.append({
            "x": xi,
            "wb": wbc.astype(bf16),
            "tb": tbc,
            "ident": ident_bf,
        })
    return in_maps


def _unpermute(dev_out):
    """Invert the PSUM column order: flat = m*2048 + h*1024 + s*128 + q
    maps to row m*2048 + q*16 + 8h + s."""
    a = dev_out.reshape(128, NMAC, 2, 8, 128)  # [p, m, h, s, q]
    a = a.transpose(0, 1, 4, 2, 3)  # [p, m, q, h, s]
    return np.ascontiguousarray(a).reshape(128, ROWS)


def _run(in_maps, trace=False):
    from concourse.bass_utils import run_bass_kernel_spmd
    global _LAST_RESULTS
    nc = _get_nc()
    res = run_bass_kernel_spmd(nc, in_maps, list(range(N_CORES)),
                               trace=trace)
    _LAST_RESULTS = res
    return res


def kernel(x, ln_gamma, ln_beta, W, b):
    x = np.asarray(x, np.float32)
    ln_gamma = np.asarray(ln_gamma, np.float32)
    ln_beta = np.asarray(ln_beta, np.float32)
    W = np.asarray(W, np.float32)
    b = np.asarray(b, np.float32)

    in_maps = _make_in_maps(x, ln_gamma, ln_beta, W, b)
    res = _run(in_maps, trace=False)
    outs = [_unpermute(np.asarray(r["out"])).astype(np.float32)
            for r in res.results]
    full = np.concatenate(outs, axis=0)  # [1024, 32768]
    return full.reshape(1024, 8, 64, 64)
